# revision 27
# baseline (speedup 1.0000x reference)
"""Trainium2 Bass kernel for nn_MultiHeadedAttention — fp8 DoubleRow v4.

Scores ride ONE fp8e4m3 DoubleRow matmul per (head, k-tile): contraction
K=65 x 2 = 130 slots hold the 4 cross products of a hi/lo fp8 split of
both sides (x = x_hi + x_lo, each e4m3; rel err ~0.1% per entry) plus the
Schraudolph aux slot. Cost model: DoubleRow = 0.5 cycles/output-row vs 1
for f32r, and cost is independent of contraction depth, so the hi/lo
split and the aux row are free on the PE.

Slot map (partition p, DoubleRow half t):
  p in [0,32):  q side q_hi[d=p] (t-broadcast, stride-0), k side (k_hi, k_lo)
  p in [32,64): q side q_lo[d],                            k side (k_hi, k_lo)
  p = 64:       q side aux_q (both t),                     k side (1, 0)
The psum result is ps' = (A*(s - m) + B)/DIV in scaled Schraudolph space
(DIV=128 keeps aux inside e4m3 range; aux precision only shifts num and
den together, so it cancels exactly in num/den).

exp + mask on two engines, alternating per chunk:
  a: ACT exp (scale=DIV/A undoes the prescale) -> Pool bf16 mask-mult
  d: ONE DVE scalar_tensor_tensor: u16 = (ps' * DIV) * mask — saturating
     u16 cast clamps negatives to +0.0 bits, so the u16 output IS the bf16
     bit pattern of 2^((ps-B)/128) ~ exp(s-m) with mask folded in.
Softmax num/den ride the PE as before: [num; den] = [v|1]^T @ et, bf16.
num/den go to DRAM; division and head-mean are host-side.

Projection biases are folded into the ACT activations (per-partition bias
operand) instead of rank-1 bias matmuls. The per-query shift enters via
the fp8 aux slot, so the old aux-row injection matmuls are gone.
Sharding: core c -> batch b=c//2, query-half c%2.
"""

import numpy as np

import concourse.mybir as mybir
from concourse import bacc
from concourse.tile import TileContext
from concourse import bass_utils

F32 = mybir.dt.float32
F32R = mybir.dt.float32r
BF16 = mybir.dt.bfloat16
F8 = mybir.dt.float8e4
U16 = mybir.dt.uint16

B, SQ, SK, D, H, DK = 4, 4096, 4096, 256, 8, 32
NCORES = 8
R = SQ // 2          # q rows per core
QH = R // 1024       # 2 q-half blocks of 1024
KT = SK // 128       # 32 k-tiles of 128
SCALE = 10.0 / (32.0 ** 0.25)
LAM = 1.51           # shift coefficient, window [1.36, 1.66]
A16 = 128.0 / np.log(2.0)          # schraudolph scale (bf16-bits space)
C16 = -7.5                         # schraudolph bias correction
B16 = 127.0 * 128.0 + C16
DIV = 128.0                        # fp8-space divisor: ps' = ps/DIV
ALPHA = float(np.sqrt((A16 / DIV) * SCALE * SCALE))  # per-side dir scale

ROTS = [(0, 1), (2, 3), (4, 5), (6, 7)]
# per-half-tile route: d = fused DVE schraudolph, a = ACT exp + Pool mult,
# g = ACT exp + DVE mult. d:a:g = 15:14:3 balances ACT/DVE/Pool under PE.
ROUTE = list('dadadgdadadadada') + list('dgdadadadadgdaaa')

_CACHE = {}


def _build(repeat=1):
    if repeat in _CACHE:
        return _CACHE[repeat]
    nc = bacc.Bacc("TRN2", target_bir_lowering=False, debug=False,
                   num_devices=NCORES)

    qT_d = nc.dram_tensor("qT", [D, R], F32, kind="ExternalInput")
    kT_d = nc.dram_tensor("kT", [D, SK], F32, kind="ExternalInput")
    v_d = nc.dram_tensor("v", [1, SK], F32, kind="ExternalInput")
    mt_d = nc.dram_tensor("mt", [SK, R], BF16, kind="ExternalInput")
    w0p_d = nc.dram_tensor("w0p", [D, 4 * 128], F32, kind="ExternalInput")
    w1t8_d = nc.dram_tensor("w1t8", [D, H], F32, kind="ExternalInput")
    b0c_d = nc.dram_tensor("b0c", [128, 4], F32, kind="ExternalInput")
    b18c_d = nc.dram_tensor("b18c", [8, 2], F32, kind="ExternalInput")
    inds_d = nc.dram_tensor("inds", [128, 4 * H], F32, kind="ExternalInput")
    indst_d = nc.dram_tensor("indst", [H, 4 * 128], F32, kind="ExternalInput")
    kaux_d = nc.dram_tensor("kaux", [1, H * SK], F8, kind="ExternalInput")
    out_d = nc.dram_tensor("o", [8, QH * 2048], F32, kind="ExternalOutput")

    def mm(out, lhsT, rhs, **kw):
        nc.tensor.matmul(out, lhsT.bitcast(F32R), rhs.bitcast(F32R), **kw)

    phases = [(rep, qh) for rep in range(repeat) for qh in range(QH)]

    with TileContext(nc) as tc:
        with tc.tile_pool(name="persist", bufs=1) as pp, \
             tc.tile_pool(name="maskpA", bufs=1) as maskpA:
            w1t8 = pp.tile([128, 2, H], F32, tag="w1t8")
            nc.gpsimd.dma_start(w1t8[:].bitcast(F32R),
                                w1t8_d.rearrange("(a p) o -> p a o",
                                                 p=128).bitcast(F32R))
            b18c = pp.tile([8, 2], F32, tag="b18c")  # col0: b1, col1: SCALE*b1
            nc.gpsimd.dma_start(b18c[:].bitcast(F32R), b18c_d[:].bitcast(F32R))
            b0c = pp.tile([128, 4], F32, tag="b0c")
            nc.gpsimd.dma_start(b0c[:].bitcast(F32R), b0c_d[:].bitcast(F32R))
            expbias = pp.tile([128, 1], F32, tag="expbias")
            nc.gpsimd.memset(expbias[:], -B16 / A16)

            # fp8 score operands (DoubleRow layout, see module docstring)
            kdT8 = pp.tile([65, 2, H, SK], F8, tag="kdT8")
            qdT8 = pp.tile([65, H, R], F8, tag="qdT8")
            # aux k-side slot: (64, t0) = 1.0 via DRAM constant (a memset of a
            # single-partition row costs ~27us of engine time); (64, t1) = 0
            # comes from the lo-relayout DMA reading one zero-padding row.

            # mask quarters: tag j covers k-tiles 8j..8j+7 of one query-half.
            mask_tiles = {}
            mask_pools = {0: maskpA}

            def ensure_mask(ph, j):
                if ph >= len(phases) or (ph, j) in mask_tiles:
                    return
                rep, qh = phases[ph]
                q0 = qh * 1024
                t = mask_pools[j].tile([128, 8, 1024], BF16, tag=f"mq{j}")
                k0 = j * 8 * 128
                nc.sync.dma_start(
                    t[:],
                    mt_d[k0:k0 + 1024, q0:q0 + 1024].rearrange(
                        "(c p) q -> p c q", p=128))
                mask_tiles[(ph, j)] = t

            shp_ctx = tc.tile_pool(name="shp", bufs=1)
            shp = shp_ctx.__enter__()
            sskp = shp.tile([8, 8], F32, tag="sskp")     # per-chunk sum kn'^2
            ssk = shp.tile([8, 1], F32, tag="ssk")
            tsh = shp.tile([8, 1], F32, tag="tsh")
            tshA = shp.tile([8, 1], F32, tag="tshA")     # tsh * -A16/DIV
            b16bc = shp.tile([8, 512], F32, tag="b16bc")
            nc.gpsimd.memset(b16bc[:], B16 / DIV)
            mq = None                          # allocated after the k pass
            aux8 = None

            def project(src_d, rows, pfx, is_q):
                nch = rows // 512
                with (
                    tc.tile_pool(name=pfx + "xT", bufs=1) as xTp,
                    tc.tile_pool(name=pfx + "psP", bufs=3, space="PSUM") as psP,
                    tc.tile_pool(name=pfx + "psS", bufs=1, space="PSUM") as psS,
                    tc.tile_pool(name=pfx + "psE", bufs=3, space="PSUM") as psE,
                    tc.tile_pool(name=pfx + "sq", bufs=1) as sqp,
                    tc.tile_pool(name=pfx + "sm", bufs=2) as smp,
                    tc.tile_pool(name=pfx + "xd", bufs=2) as xdp,
                    tc.tile_pool(name=pfx + "hilo", bufs=1) as hlp,
                    tc.tile_pool(name=pfx + "cst", bufs=1) as cstp,
                ):
                    w0p = cstp.tile([128, 2, 4, 128], F32, tag=pfx + "w0p")
                    nc.sync.dma_start(
                        w0p[:].bitcast(F32R),
                        w0p_d.rearrange("(a p) (g o) -> p a g o",
                                        p=128, g=4).bitcast(F32R))
                    inds = cstp.tile([128, 4, H], BF16, tag=pfx + "inds")
                    nc.gpsimd.dma_start(
                        inds[:], inds_d.rearrange("p (g o) -> p g o", g=4))
                    indst = cstp.tile([H, 4, 128], F32, tag=pfx + "indst")
                    nc.gpsimd.dma_start(
                        indst[:].bitcast(F32R),
                        indst_d.rearrange("p (g o) -> p g o", g=4).bitcast(F32R))
                    # fp8 hi/lo staging, one 2048-column stage at a time
                    xT = xTp.tile([128, 2, rows], F32, tag="xT")
                    hw = rows // 2
                    for xh in range(2):
                        nc.sync.dma_start(
                            xT[:, :, xh * hw:(xh + 1) * hw].bitcast(F32R),
                            src_d[:, xh * hw:(xh + 1) * hw].rearrange(
                                "(kc p) r -> p kc r", p=128).bitcast(F32R))
                    if not is_q:
                        nc.sync.dma_start(
                            kdT8[64:65, 0].rearrange("a h k -> a (h k)"),
                            kaux_d[:])
                    hi8 = lo8 = None

                    def relayout(stg):
                        ks = slice(stg * 2048, (stg + 1) * 2048)
                        if is_q:
                            for u in range(2):
                                s = slice(64 * u, 64 * u + 32)
                                nc.sync.dma_start(qdT8[0:32, u::2, ks],
                                                  hi8[s])
                                nc.gpsimd.dma_start(qdT8[32:64, u::2, ks],
                                                    lo8[s])
                        else:
                            for u in range(2):
                                s = slice(64 * u, 64 * u + 32)
                                s33 = slice(64 * u, 64 * u + 33)
                                nc.sync.dma_start(
                                    kdT8[0:32, 0, u::2, ks], hi8[s])
                                nc.sync.dma_start(
                                    kdT8[32:64, 0, u::2, ks], hi8[s])
                                nc.gpsimd.dma_start(
                                    kdT8[0:32, 1, u::2, ks], lo8[s])
                                # 33 rows: row 64u+32 is zero padding and
                                # lands on the aux row (64, t1) = 0
                                nc.gpsimd.dma_start(
                                    kdT8[32:65, 1, u::2, ks], lo8[s33])

                    for ch in range(nch):
                        cs = slice(ch * 512, (ch + 1) * 512)
                        css = slice((ch % 4) * 512, (ch % 4 + 1) * 512)
                        if ch % 4 == 0:
                            hi8 = hlp.tile([128, 4, 2048], F8,
                                           tag=pfx + "hi8", name=f"hi8_{ch}")
                            lo8 = hlp.tile([128, 4, 2048], F8,
                                           tag=pfx + "lo8", name=f"lo8_{ch}")
                        if not is_q:
                            if ch == 3:
                                ensure_mask(0, 0)
                        # norms projection qn[8, 512] (bias via ACT)
                        pn = psS.tile([8, 512], F32, tag="pn")
                        for kc in range(2):
                            mm(pn[:], w1t8[:, kc, :], xT[:, kc, cs],
                               start=(kc == 0), stop=(kc == 1))
                        if is_q:
                            # mq = SCALE*|qn| = |SCALE*pn + SCALE*b1|
                            nc.scalar.activation(
                                mq[:, cs], pn[:],
                                mybir.ActivationFunctionType.Abs,
                                bias=b18c[:, 1:2], scale=SCALE)
                        else:
                            sqn = smp.tile([8, 512], F32, tag="sqn")
                            nc.scalar.activation(
                                sqn[:], pn[:],
                                mybir.ActivationFunctionType.Square,
                                bias=b18c[:, 0:1], scale=1.0)
                            nc.vector.tensor_reduce(
                                sskp[:, ch:ch + 1], sqn[:],
                                axis=mybir.AxisListType.X,
                                op=mybir.AluOpType.add)
                        # per-group direction projections + scaling.
                        # gp 0/1: rw = pr + b0 materialized (frees pr early);
                        # gp 2/3: bias folded into the final xd stt (pr must
                        # stay live, psP has exactly the banks for it).
                        sq_ = [None] * 4
                        rw_ = [None] * 4
                        pr_ = [None] * 4
                        for gp in range(4):
                            pr = psP.tile([128, 512], F32, tag="pr",
                                          name=f"pr{gp}")
                            pr_[gp] = pr
                            for kc in range(2):
                                mm(pr[:], w0p[:, kc, gp, :], xT[:, kc, cs],
                                   start=(kc == 0), stop=(kc == 1))
                            sq_[gp] = sqp.tile([128, 512], BF16,
                                               tag=f"sq{gp}", name=f"sq{gp}")
                            nc.scalar.activation(
                                sq_[gp][:], pr[:],
                                mybir.ActivationFunctionType.Square,
                                bias=b0c[:, gp:gp + 1], scale=1.0)
                            if gp < 2:
                                rw_[gp] = sqp.tile([128, 512], F32,
                                                   tag=f"rw{gp}",
                                                   name=f"rw{gp}")
                                if gp == 0:
                                    nc.scalar.activation(
                                        rw_[gp][:], pr[:],
                                        mybir.ActivationFunctionType.Identity,
                                        bias=b0c[:, gp:gp + 1], scale=1.0)
                                else:
                                    nc.vector.tensor_scalar(
                                        out=rw_[gp][:], in0=pr[:],
                                        scalar1=b0c[:, gp:gp + 1],
                                        scalar2=0.0,
                                        op0=mybir.AluOpType.add,
                                        op1=mybir.AluOpType.add)
                        pss = psS.tile([8, 512], F32, tag="pss")
                        for gp in range(4):
                            nc.tensor.matmul(pss[:], inds[:, gp, :],
                                             sq_[gp][:],
                                             start=(gp == 0), stop=(gp == 3))
                        srt = smp.tile([8, 512], F32, tag="srt")
                        nc.scalar.activation(
                            srt[:], pss[:],
                            mybir.ActivationFunctionType.Sqrt,
                            scale=1.0 / (ALPHA * ALPHA))
                        rn = smp.tile([8, 512], F32, tag="rn")
                        nc.vector.reciprocal_approx_fast(rn[:], srt[:])
                        av = smp.tile([8, 512], F32, tag="av")
                        nc.vector.scalar_tensor_tensor(
                            out=av[:].bitcast(F32R), in0=pn[:],
                            scalar=b18c[:, 0:1], in1=rn[:],
                            op0=mybir.AluOpType.add,
                            op1=mybir.AluOpType.mult)
                        if is_q:
                            # aux8 = (B16 - A16*mq*tsh)/DIV, cast e4m3
                            nc.vector.scalar_tensor_tensor(
                                out=aux8[:, cs], in0=mq[:, cs],
                                scalar=tshA[:], in1=b16bc[:],
                                op0=mybir.AluOpType.mult,
                                op1=mybir.AluOpType.add)
                        xd = xdp.tile([128, 4, 512], F32, tag="xd")
                        for gp in range(4):
                            pe = psE.tile([128, 512], F32, tag="pe")
                            mm(pe[:], indst[:, gp, :], av[:],
                               start=True, stop=True)
                            if gp < 2:
                                nc.vector.tensor_mul(
                                    xd[:, gp], rw_[gp][:], pe[:])
                            else:
                                nc.vector.scalar_tensor_tensor(
                                    out=xd[:, gp], in0=pr_[gp][:],
                                    scalar=b0c[:, gp:gp + 1], in1=pe[:],
                                    op0=mybir.AluOpType.add,
                                    op1=mybir.AluOpType.mult)
                        # hi/lo e4m3 split on Pool
                        nc.gpsimd.tensor_scalar(
                            out=hi8[:, :, css], in0=xd[:], scalar1=1.0,
                            scalar2=0.0, op0=mybir.AluOpType.mult,
                            op1=mybir.AluOpType.add)
                        nc.gpsimd.tensor_tensor(
                            out=lo8[:, :, css], in0=xd[:], in1=hi8[:, :, css],
                            op=mybir.AluOpType.subtract)
                        if ch % 4 == 3:
                            relayout(ch // 4)
                    if is_q:
                        for h in range(H):
                            nc.sync.dma_start(qdT8[64:65, h, :],
                                              aux8[h:h + 1, :])

            # k-projection first: tsh is then ready before the q-projection.
            project(kT_d, SK, "k", False)

            # shift scale: tsh = LAM*SCALE^2*RMS(kn); tshA = -A16*tsh/DIV
            nc.vector.tensor_reduce(ssk[:], sskp[:],
                                    axis=mybir.AxisListType.X,
                                    op=mybir.AluOpType.add)
            nc.scalar.activation(tsh[:], ssk[:],
                                 mybir.ActivationFunctionType.Sqrt,
                                 scale=LAM * LAM * SCALE * SCALE / float(SK))
            nc.scalar.mul(tshA[:], tsh[:], -A16 / DIV)

            # q-side scratch
            mq = shp.tile([8, R], F32, tag="mq")         # SCALE*|qn|
            aux8 = shp.tile([8, R], F8, tag="aux8")
            uvt = pp.tile([128, KT, 2], BF16, tag="uvt")
            nc.gpsimd.dma_start(uvt[:, :, 0],
                                v_d.rearrange("a (c p) -> p (a c)", p=128))
            nc.gpsimd.memset(uvt[:, :, 1:2], 1.0)

            project(qT_d, R, "q", True)

            shp_ctx.__exit__(None, None, None)

            # ---- main attention loop ----
            with (
                tc.tile_pool(name="maskpB", bufs=1) as maskpB,
                tc.tile_pool(name="psSc", bufs=6, space="PSUM") as psc,
                tc.tile_pool(name="psNd", bufs=1, space="PSUM") as psnd,
                tc.tile_pool(name="eraw", bufs=4) as erawp,
                tc.tile_pool(name="etl", bufs=8) as etlp,
                tc.tile_pool(name="ndsb", bufs=2) as ndsbp,
            ):
                for _j in (1, 2, 3):
                    mask_pools[_j] = maskpB
                for ph, (rep, qh) in enumerate(phases):
                    q0 = qh * 1024
                    for j in range(4):
                        ensure_mask(ph, j)
                    for ri, rot in enumerate(ROTS):
                        nb = 0
                        ndall = psnd.tile([64, 1024], F32, tag="ndall")
                        last_rot = ri == len(ROTS) - 1
                        for kc in range(KT):
                            msl = mask_tiles[(ph, kc // 8)][:, kc % 8, :]
                            for slot, h in enumerate(rot):
                                co = nb + 32 * slot
                                kv = kdT8[0:65, :, h,
                                          kc * 128:(kc + 1) * 128]
                                for j in range(2):
                                    js = slice(j * 512, (j + 1) * 512)
                                    ps = psc.tile([128, 512], F32, tag="ps",
                                                  name=f"ps{slot}{j}")
                                    qv = qdT8[0:65, h,
                                              q0 + j * 512:q0 + (j + 1) * 512]
                                    nc.tensor.matmul(
                                        ps[:], kv,
                                        qv.unsqueeze(1).broadcast_to(
                                            [65, 2, 512]),
                                        start=True, stop=True,
                                        perf_mode=(
                                            mybir.MatmulPerfMode.DoubleRow),
                                        tile_position=(0, 0))
                                    et = etlp.tile([128, 512], BF16,
                                                   tag="et")
                                    typ = ROUTE[(kc * 4 + slot * 2 + j)
                                                % len(ROUTE)]
                                    if typ == 'd':
                                        # d: fused DVE schraudolph (u16 sat)
                                        nc.vector.scalar_tensor_tensor(
                                            out=et[:].bitcast(U16),
                                            in0=ps[:], scalar=DIV,
                                            in1=msl[:, js],
                                            op0=mybir.AluOpType.mult,
                                            op1=mybir.AluOpType.mult)
                                    else:
                                        # a/g: ACT exp -> Pool/DVE mask-mult
                                        er = erawp.tile([128, 512], BF16,
                                                        tag="er")
                                        nc.scalar.activation(
                                            er[:], ps[:],
                                            mybir.ActivationFunctionType.Exp,
                                            bias=expbias[:], scale=DIV / A16)
                                        eng = (nc.gpsimd if typ == 'a'
                                               else nc.vector)
                                        eng.tensor_tensor(
                                            out=et[:], in0=er[:],
                                            in1=msl[:, js],
                                            op=mybir.AluOpType.mult)
                                    nc.tensor.matmul(
                                        ndall[co:co + 2,
                                              j * 512:(j + 1) * 512],
                                        uvt[:, kc, :], et[:],
                                        start=(kc == 0), stop=(kc == KT - 1),
                                        tile_position=(0, co))
                            if last_rot and kc % 8 == 7:
                                ensure_mask(ph + 1, kc // 8)
                        ndsb = ndsbp.tile([34, 1024], F32, tag="ndsb")
                        nc.scalar.copy(ndsb[:], ndall[0:34, :])
                        for slot, h in enumerate(rot):
                            nc.sync.dma_start(
                                out_d[h:h + 1, qh * 2048:(qh + 1) * 2048],
                                ndsb[32 * slot:32 * slot + 2, :])

    nc.finalize()
    _CACHE[repeat] = nc
    return nc


def _prep_host(query, key, value, mask, w0, b0, w1, b1):
    import ml_dtypes
    # outc permutation: group gp = h//2 holds head 2gp at rows 0-31 and head
    # 2gp+1 at rows 64-95; rows 32-63/96-127 are zero padding.
    w0p = np.zeros((D, 4 * 128), np.float32)
    b0c = np.zeros((128, 4), np.float32)
    inds = np.zeros((128, 4 * H), np.float32)
    indst = np.zeros((H, 4 * 128), np.float32)
    w0t = w0.T.astype(np.float32)            # [inc, outc]
    for h in range(H):
        gp, u = divmod(h, 2)
        dst = gp * 128 + 64 * u
        w0p[:, dst:dst + 32] = w0t[:, 32 * h:32 * h + 32]
        b0c[64 * u:64 * u + 32, gp] = b0[32 * h:32 * h + 32]
        inds[64 * u:64 * u + 32, gp * H + h] = 1.0
        indst[h, gp * 128 + 64 * u:gp * 128 + 64 * u + 32] = 1.0
    w1t8 = np.ascontiguousarray(w1[:H].T.astype(np.float32))
    b18c = np.stack([b1[:H], SCALE * b1[:H]], axis=1).astype(np.float32)
    kaux = np.ones((1, H * SK), ml_dtypes.float8_e4m3)
    in_maps = []
    for c in range(NCORES):
        b, half = divmod(c, 2)
        r0 = half * R
        mt = np.ascontiguousarray(mask[b, r0:r0 + R].T).astype(
            ml_dtypes.bfloat16)
        in_maps.append({
            "qT": np.ascontiguousarray(query[b, r0:r0 + R].T),
            "kT": np.ascontiguousarray(key[b].T),
            "v": np.ascontiguousarray(value[b].reshape(1, SK)),
            "mt": mt,
            "w0p": w0p, "w1t8": w1t8, "b0c": b0c, "b18c": b18c,
            "inds": inds, "indst": indst, "kaux": kaux,
        })
    return in_maps


def kernel(query, key, value, mask, w0, b0, w1, b1, _repeat=1):
    query = np.asarray(query, np.float32)
    key = np.asarray(key, np.float32)
    value = np.asarray(value, np.float32)
    mask = np.asarray(mask, np.int32)
    nc = _build(_repeat)
    in_maps = _prep_host(query, key, value, mask, w0, b0, w1, b1)
    res = bass_utils.run_bass_kernel_spmd(nc, in_maps,
                                          core_ids=list(range(NCORES)))
    out = np.empty((B, SQ, 1), np.float32)
    for c in range(NCORES):
        b, half = divmod(c, 2)
        o = res.results[c]["o"].reshape(8, QH, 2048)
        x = o[:, :, 0:1024] / o[:, :, 1024:2048]
        out[b, half * R:(half + 1) * R, 0] = x.reshape(8, R).mean(axis=0)
    return out
